# revision 1
# baseline (speedup 1.0000x reference)
"""Fused conv-attention kernel for Trainium2, sharded over 8 NeuronCores.

Reference computation (B=2, H=12, L=T=1024, D=64, FEA=3, DIM=768):
    scores = concat([s0,s1,s2], ch)            # [b, 36, l, t]
    fused  = einsum('bclt,oc->bolt', scores, fuse_w) + fuse_b
    attn   = softmax(fused, axis=-1)
    x      = einsum('bhlt,bhtd->bhld', attn, v)
    y      = merge_heads(x) @ proj_w.T + proj_b  # [b, l, 768]

Sharding: fully data-parallel over (b, l-block): core k handles b=k//4 and
l-rows [256*(k%4), 256*(k%4)+256).  Every op is local; no collectives.

Per-core dataflow:
  - conv as block-diag matmul: 8 l-rows per group, lhsT_j [96,96] holds
    fuse_w columns for score tensor j replicated block-diagonally, so one
    matmul computes 8 l-rows x 12 heads at once (K=96, M=96, N=512).
  - exp via ScalarE activation (bias=fuse_b, accum_out=row sums); softmax max
    subtraction is skipped (|fused| <= ~5 so exp is safe in fp32).
  - normalize by 1/rowsum on VectorE, transpose 128-col chunks on PE into
    attn^T [t, (head, l)] layout, gather with one strided DVE copy.
  - attn @ V as out[d, l] = v^T-stationary matmuls (K=t tiles of 128, N=256).
  - final proj as out[l, 768] = x^T-stationary matmuls, bias added by DVE.
Matmul operands are bitcast to float32r (fp32 data, 4x PE throughput).
"""

import os
import sys

import numpy as np

sys.path.insert(0, "/opt/trn_rl_repo")

B, H, L, T, D = 2, 12, 1024, 1024, 64
DIM = H * D  # 768
NCORES = 8
LC = L * B // NCORES  # 256 l-rows per core
G = 8  # l-rows per conv group
NG = LC // G  # 32 groups
KM = 12 * G  # 96: conv matmul K and M
NTT = T // 128  # 8 t-tiles

_CACHE = {}


def _build_nc():
    import concourse.bacc as bacc
    import concourse.bass as bass
    import concourse.mybir as mybir
    import concourse.tile as tile
    from concourse.masks import make_identity
    from contextlib import ExitStack

    f32 = mybir.dt.float32
    f32r = mybir.dt.float32r

    nc = bacc.Bacc(
        "TRN2", target_bir_lowering=False, debug=False, enable_asserts=False
    )

    s_in = [
        nc.dram_tensor(f"s{j}c", [12, LC, T], f32r, kind="ExternalInput").ap()
        for j in range(3)
    ]
    v_in = nc.dram_tensor("vc", [H, T, D], f32r, kind="ExternalInput").ap()
    w_in = [
        nc.dram_tensor(f"w{j}", [KM, KM], f32r, kind="ExternalInput").ap()
        for j in range(3)
    ]
    b_in = nc.dram_tensor("b96", [KM, 1], f32, kind="ExternalInput").ap()
    pw_in = nc.dram_tensor("pwT", [DIM, DIM], f32r, kind="ExternalInput").ap()
    pb_in = nc.dram_tensor("pbb", [128, DIM], f32, kind="ExternalInput").ap()
    out_d = nc.dram_tensor("out", [LC, DIM], f32, kind="ExternalOutput").ap()

    def r(ap):  # fp32 -> float32r view for matmul operands
        return ap.bitcast(f32r)

    with tile.TileContext(nc) as tc, ExitStack() as ctx:
        # ---- persistent SBUF ----
        singles = ctx.enter_context(tc.tile_pool(name="singles", bufs=1))
        ident = singles.tile([128, 128], f32)
        make_identity(nc, ident[:])
        wt = [singles.tile([KM, KM], f32r, tag=f"wt{j}", name=f"wt{j}") for j in range(3)]
        for j in range(3):
            nc.sync.dma_start(wt[j][:], w_in[j])
        b96 = singles.tile([KM, 1], f32)
        nc.sync.dma_start(b96[:], b_in)
        pw = singles.tile([128, 6 * DIM], f32r)  # [i-tile part, ki*768+o]
        for ki in range(6):
            nc.sync.dma_start(
                pw[:, ki * DIM : (ki + 1) * DIM], pw_in[ki * 128 : (ki + 1) * 128, :]
            )
        pb = singles.tile([128, DIM], f32)
        nc.sync.dma_start(pb[:], pb_in)
        vsb = singles.tile([128, H * NTT * D], f32r)  # [t-part, h*512 + tt*64 + d]
        for h in range(H):
            nc.sync.dma_start(
                vsb[:, h * 512 : (h + 1) * 512]
                .rearrange("p (tt d) -> p tt d", tt=NTT),
                v_in[h].rearrange("(tt p) d -> p tt d", p=128),
            )
        # attn^T accumulator: [t-part(128) , tt*3072 + h*256 + l]
        attnT = singles.tile([128, NTT * H * LC], f32r)
        # x^T for proj: [i%128 part, (i//128)*256 + l]
        xT = singles.tile([128, 6 * LC], f32r)

        # ---- phase 1: conv + softmax + transpose, per group of 8 l-rows ----
        with ExitStack() as p1:
            spool = p1.enter_context(tc.tile_pool(name="scores", bufs=3))
            fpsum = p1.enter_context(
                tc.tile_pool(name="fpsum", bufs=2, space="PSUM")
            )
            epool = p1.enter_context(tc.tile_pool(name="exp", bufs=3))
            zpool = p1.enter_context(tc.tile_pool(name="z", bufs=4))
            tpsum = p1.enter_context(
                tc.tile_pool(name="tpsum", bufs=4, space="PSUM")
            )
            for g in range(NG):
                st = [spool.tile([KM, T], f32r, tag=f"st{j}", name=f"st{j}_{g}") for j in range(3)]
                for j in range(3):
                    nc.sync.dma_start(
                        st[j][:],
                        s_in[j][:, g * G : (g + 1) * G, :].rearrange(
                            "c lg t -> lg c t"
                        ),
                    )
                fp = fpsum.tile([KM, T], f32)
                for th in range(2):
                    for j in range(3):
                        nc.tensor.matmul(
                            fp[:, th * 512 : (th + 1) * 512],
                            wt[j][:],
                            st[j][:, th * 512 : (th + 1) * 512],
                            start=(j == 0),
                            stop=(j == 2),
                        )
                et = epool.tile([KM, T], f32)
                zt = zpool.tile([KM, 1], f32, tag="zt")
                nc.scalar.activation(
                    et[:],
                    fp[:],
                    mybir.ActivationFunctionType.Exp,
                    bias=b96[:],
                    accum_out=zt[:],
                )
                zi = zpool.tile([KM, 1], f32, tag="zi")
                nc.vector.reciprocal(zi[:], zt[:])
                nc.vector.tensor_scalar_mul(et[:], et[:], zi[:])
                for tt in range(NTT):
                    tp = tpsum.tile([128, KM], f32)
                    nc.tensor.transpose(
                        tp[:], et[:, tt * 128 : (tt + 1) * 128], ident[:KM, :KM]
                    )
                    dst = (
                        attnT[:]
                        .rearrange("p (tt h l) -> p tt h l", tt=NTT, h=H)[
                            :, tt, :, g * G : (g + 1) * G
                        ]
                    )
                    nc.vector.tensor_copy(
                        dst, tp[:].rearrange("p (h lg) -> p h lg", h=H)
                    )

        # ---- phase 2: attn @ V  -> x^T ----
        with ExitStack() as p2:
            xpsum = p2.enter_context(
                tc.tile_pool(name="xpsum", bufs=2, space="PSUM")
            )
            for h in range(H):
                xp = xpsum.tile([D, LC], f32)
                for tt in range(NTT):
                    nc.tensor.matmul(
                        xp[:],
                        vsb[:, h * 512 + tt * D : h * 512 + (tt + 1) * D],
                        attnT[
                            :, tt * H * LC + h * LC : tt * H * LC + (h + 1) * LC
                        ],
                        start=(tt == 0),
                        stop=(tt == NTT - 1),
                    )
                po = (h % 2) * D
                ko = (h // 2) * LC
                nc.vector.tensor_copy(xT[po : po + D, ko : ko + LC], xp[:])

            # ---- phase 3: proj -> out ----
            ppsum = p2.enter_context(
                tc.tile_pool(name="ppsum", bufs=2, space="PSUM")
            )
            ypool = p2.enter_context(tc.tile_pool(name="y", bufs=2))
            for lc in range(2):
                pp = ppsum.tile([128, 1024], f32)
                for ki in range(6):
                    lhs = xT[:, ki * LC + lc * 128 : ki * LC + (lc + 1) * 128]
                    nc.tensor.matmul(
                        pp[:, 0:512],
                        lhs,
                        pw[:, ki * DIM : ki * DIM + 512],
                        start=(ki == 0),
                        stop=(ki == 5),
                    )
                    nc.tensor.matmul(
                        pp[:, 512:768],
                        lhs,
                        pw[:, ki * DIM + 512 : ki * DIM + DIM],
                        start=(ki == 0),
                        stop=(ki == 5),
                    )
                yt = ypool.tile([128, DIM], f32)
                nc.vector.tensor_add(yt[:], pp[:, 0:DIM], pb[:])
                nc.sync.dma_start(out_d[lc * 128 : (lc + 1) * 128, :], yt[:])

    nc.compile()
    return nc


def _host_prep(s0, s1, s2, v, fuse_w, fuse_b, proj_w, proj_b):
    """Build per-core input maps."""
    s0 = np.asarray(s0, dtype=np.float32)
    s1 = np.asarray(s1, dtype=np.float32)
    s2 = np.asarray(s2, dtype=np.float32)
    v = np.asarray(v, dtype=np.float32)
    fuse_w = np.asarray(fuse_w, dtype=np.float32)
    fuse_b = np.asarray(fuse_b, dtype=np.float32)
    proj_w = np.asarray(proj_w, dtype=np.float32)
    proj_b = np.asarray(proj_b, dtype=np.float32)

    # block-diag conv weights: w_j[k=(lg,c), m=(o,lg)] = fuse_w[o, 12j+c] @ lg==lg'
    ws = []
    for j in range(3):
        wj = np.zeros((KM, KM), dtype=np.float32)
        blk = fuse_w[:, 12 * j : 12 * (j + 1)].T  # [c, o]
        for lg in range(G):
            # rows lg*12..lg*12+12 (c), cols o*G+lg
            wj[lg * 12 : (lg + 1) * 12, lg::G] = blk
        ws.append(wj)
    b96 = np.repeat(fuse_b, G).astype(np.float32).reshape(KM, 1)  # p = o*G+lg
    pwT = np.ascontiguousarray(proj_w.T)
    pbb = np.broadcast_to(proj_b, (128, DIM)).copy()

    in_maps = []
    for k in range(NCORES):
        b = k // (NCORES // B)
        l0 = (k % (NCORES // B)) * LC
        m = {
            "s0c": np.ascontiguousarray(s0[b, :, l0 : l0 + LC, :]),
            "s1c": np.ascontiguousarray(s1[b, :, l0 : l0 + LC, :]),
            "s2c": np.ascontiguousarray(s2[b, :, l0 : l0 + LC, :]),
            "vc": np.ascontiguousarray(v[b]),
            "w0": ws[0],
            "w1": ws[1],
            "w2": ws[2],
            "b96": b96,
            "pwT": pwT,
            "pbb": pbb,
        }
        in_maps.append(m)
    return in_maps


def _install_ntff_hook():
    """Provide antenv.axon_hooks (absent in this image) so trace=True works."""
    try:
        from antenv import axon_hooks  # noqa: F401

        return True
    except ImportError:
        pass
    try:
        import types
        import ctypes
        import contextlib
        import antenv

        so_path = "/opt/axon/libaxon_pjrt.so"
        if not os.path.exists(so_path):
            return False
        lib = ctypes.CDLL(so_path)
        if not hasattr(lib, "axon_start_nrt_profile"):
            return False
        lib.axon_start_nrt_profile.argtypes = [
            ctypes.POINTER(ctypes.c_int64),
            ctypes.c_size_t,
        ]
        lib.axon_start_nrt_profile.restype = ctypes.c_int64
        lib.axon_stop_nrt_profile.argtypes = [ctypes.c_char_p]
        lib.axon_stop_nrt_profile.restype = ctypes.c_int64

        @contextlib.contextmanager
        def _hook(output_dir, device_ids):
            import jax

            jax.devices()
            if device_ids:
                ids = (ctypes.c_int64 * len(device_ids))(*device_ids)
                rc = lib.axon_start_nrt_profile(ids, len(device_ids))
            else:
                rc = lib.axon_start_nrt_profile(None, 0)
            if rc != 0:
                raise RuntimeError(f"axon_start_nrt_profile rc={rc}")
            try:
                yield
            finally:
                n = lib.axon_stop_nrt_profile(str(output_dir).encode())
                print(f"ntff profile: {n} file(s) -> {output_dir}", file=sys.stderr)

        mod = types.ModuleType("antenv.axon_hooks")
        _h = {"hook": _hook}
        mod.set_axon_ntff_profile_hook = lambda h: _h.__setitem__("hook", h)
        mod.get_axon_ntff_profile_hook = lambda: _h["hook"]
        sys.modules["antenv.axon_hooks"] = mod
        antenv.axon_hooks = mod
        return True
    except Exception as e:  # degrade to untraced
        print("ntff hook install failed:", e, file=sys.stderr)
        return False


def kernel(s0, s1, s2, v, fuse_w, fuse_b, proj_w, proj_b, _trace=False):
    from concourse import bass_utils
    from concourse.bass_utils import run_bass_kernel_spmd

    if "nc" not in _CACHE:
        _CACHE["nc"] = _build_nc()
    nc = _CACHE["nc"]

    in_maps = _host_prep(s0, s1, s2, v, fuse_w, fuse_b, proj_w, proj_b)
    if _trace:
        _trace = _install_ntff_hook()
        bass_utils.upload_artifacts = lambda tmpdir: f"local:{tmpdir}"
    tmpdir = None
    if _trace:
        import tempfile

        tmpdir = tempfile.mkdtemp(prefix="bass_trace_")
        _CACHE["trace_dir"] = tmpdir
    try:
        res = run_bass_kernel_spmd(
            nc, in_maps, core_ids=list(range(NCORES)), trace=_trace, tmpdir=tmpdir
        )
    except Exception:
        if not _trace:
            raise
        import traceback

        traceback.print_exc()
        print("trace run failed; retrying untraced", file=sys.stderr)
        res = run_bass_kernel_spmd(nc, in_maps, core_ids=list(range(NCORES)))
    _CACHE["last_exec_time_ns"] = res.exec_time_ns
    _CACHE["last_results"] = res

    out = np.empty((B, L, DIM), dtype=np.float32)
    for k in range(NCORES):
        b = k // (NCORES // B)
        l0 = (k % (NCORES // B)) * LC
        out[b, l0 : l0 + LC, :] = res.results[k]["out"]
    return out



# revision 2
# speedup vs baseline: 1.7913x; 1.7913x over previous
"""Fused conv-attention kernel for Trainium2, sharded over 8 NeuronCores.

Reference computation (B=2, H=12, L=T=1024, D=64, FEA=3, DIM=768):
    scores = concat([s0,s1,s2], ch)            # [b, 36, l, t]
    fused  = einsum('bclt,oc->bolt', scores, fuse_w) + fuse_b
    attn   = softmax(fused, axis=-1)
    x      = einsum('bhlt,bhtd->bhld', attn, v)
    y      = merge_heads(x) @ proj_w.T + proj_b  # [b, l, 768]

Sharding: fully data-parallel over (b, l-block): core k handles b=k//4 and
l-rows [256*(k%4), 256*(k%4)+256).  Every op is local; no collectives.

All HBM traffic is bf16 (host casts/packs): scores 19.2MB/core with 2KB
descriptors so the DMA stripes across all 16 engines; v/proj weights flow on
the second HWDGE queue (scalar engine) in parallel with the score stream.

Per-core dataflow (G=10 l-rows per group, KM=120 partitions, LCP=260 padded):
  - conv as block-diag matmul: lhsT_j [120,120] holds fuse_w cols for score
    tensor j replicated block-diagonally over lg; j-outer loop -> 3 stationary
    loads per group, 6 matmuls of N=512 into PSUM [120,1024].
  - exp via ScalarE activation (bias=fuse_b, accum_out=row sums z); max
    subtraction skipped (|fused| <= ~5 so fp32 exp is safe).
  - softmax normalize FUSED into the PE transpose: stationary = et t-chunk,
    moving = diag(1/z) bf16 -> psum tp[t,120] holds normalized attn^T.
  - attnT sbuf layout [t-part, (tc, g, o, lg)] bf16: psum->sbuf copies are
    contiguous; phase-2 matmul rhs reads the strided (g,lg) view (N=260).
  - attn @ V: per (h, tc) matmul accumulating x^T [64, 260] over 8 t-chunks.
  - final proj: out[l,768] = sum_hp xT_hp^T @ pw_hp, bias added by DVE.
"""

import sys

import numpy as np

sys.path.insert(0, "/opt/trn_rl_repo")

B, H, L, T, D = 2, 12, 1024, 1024, 64
DIM = H * D  # 768
NCORES = 8
LC = L * B // NCORES  # 256 l-rows per core
G = 10  # l-rows per conv group
NG = 26  # groups per core (last is padded: 260 > 256)
LCP = NG * G  # 260
KM = 12 * G  # 120: conv matmul K and M
NTC = T // 128  # 8 t-chunks

_CACHE = {}


def _build_nc():
    import concourse.bacc as bacc
    import concourse.mybir as mybir
    import concourse.tile as tile
    from contextlib import ExitStack

    f32 = mybir.dt.float32
    bf16 = mybir.dt.bfloat16

    nc = bacc.Bacc(
        "TRN2", target_bir_lowering=False, debug=False, enable_asserts=False
    )

    sc_in = nc.dram_tensor("sc", [NG, 3, KM, T], bf16, kind="ExternalInput").ap()
    v_in = nc.dram_tensor("vp", [H, 128, NTC * D], bf16, kind="ExternalInput").ap()
    w_in = nc.dram_tensor("wts", [3, KM, KM], bf16, kind="ExternalInput").ap()
    b_in = nc.dram_tensor("b120", [KM, 1], f32, kind="ExternalInput").ap()
    id_in = nc.dram_tensor("identb", [KM, KM], bf16, kind="ExternalInput").ap()
    pw_in = nc.dram_tensor("pwp", [128, 6 * DIM], bf16, kind="ExternalInput").ap()
    pb_in = nc.dram_tensor("pbb", [128, DIM], f32, kind="ExternalInput").ap()
    out_d = nc.dram_tensor("out", [LC, DIM], f32, kind="ExternalOutput").ap()

    with tile.TileContext(nc) as tc, ExitStack() as ctx:
        # ---- persistent SBUF (weights arrive on the scalar HWDGE queue) ----
        singles = ctx.enter_context(tc.tile_pool(name="singles", bufs=1))
        wt = [singles.tile([KM, KM], bf16, tag=f"wt{j}", name=f"wt{j}") for j in range(3)]
        for j in range(3):
            nc.scalar.dma_start(wt[j][:], w_in[j])
        b120 = singles.tile([KM, 1], f32)
        nc.scalar.dma_start(b120[:], b_in)
        identb = singles.tile([KM, KM], bf16)
        nc.scalar.dma_start(identb[:], id_in)
        vsb = singles.tile([128, H * NTC * D], bf16)  # [t%128, h*512 + tc*64 + d]
        for h in range(H):
            nc.scalar.dma_start(vsb[:, h * 512 : (h + 1) * 512], v_in[h])
        pw = singles.tile([128, 6 * DIM], bf16)  # [i%128, hp*768 + o]
        for q in range(3):
            nc.scalar.dma_start(
                pw[:, q * 1536 : (q + 1) * 1536], pw_in[:, q * 1536 : (q + 1) * 1536]
            )
        pb = singles.tile([128, DIM], f32)
        nc.scalar.dma_start(pb[:], pb_in)
        # attn^T accumulator: [t%128, tc*3120 + g*120 + (o*10+lg)]  bf16
        attnT = singles.tile([128, NTC * NG * KM], bf16)
        # x^T for proj: [i%128 part, (i//128)*260 + l]  bf16
        xT = singles.tile([128, 6 * LCP], bf16)

        # ---- phase 1: conv + softmax + normalized transpose, per group ----
        with ExitStack() as p1:
            spool = p1.enter_context(tc.tile_pool(name="scores", bufs=3))
            fpsum = p1.enter_context(
                tc.tile_pool(name="fpsum", bufs=2, space="PSUM")
            )
            epool = p1.enter_context(tc.tile_pool(name="exp", bufs=3))
            zpool = p1.enter_context(tc.tile_pool(name="z", bufs=4))
            dpool = p1.enter_context(tc.tile_pool(name="diag", bufs=2))
            tpsum = p1.enter_context(
                tc.tile_pool(name="tpsum", bufs=4, space="PSUM")
            )
            for g in range(NG):
                st = spool.tile([KM, 3 * T], bf16, tag="st", name=f"st_{g}")
                nc.sync.dma_start(
                    st[:].rearrange("p (j t) -> p j t", j=3),
                    sc_in[g].rearrange("j p t -> p j t"),
                )
                fp = fpsum.tile([KM, T], f32)
                for j in range(3):
                    for th in range(2):
                        nc.tensor.matmul(
                            fp[:, th * 512 : (th + 1) * 512],
                            wt[j][:],
                            st[:, j * T + th * 512 : j * T + (th + 1) * 512],
                            start=(j == 0),
                            stop=(j == 2),
                        )
                et = epool.tile([KM, T], bf16, tag="et")
                zt = zpool.tile([KM, 1], f32, tag="zt")
                nc.scalar.activation(
                    et[:],
                    fp[:],
                    mybir.ActivationFunctionType.Exp,
                    bias=b120[:],
                    accum_out=zt[:],
                )
                zi = zpool.tile([KM, 1], f32, tag="zi")
                nc.vector.reciprocal(zi[:], zt[:])
                dg = dpool.tile([KM, KM], bf16, tag="dg")
                nc.vector.tensor_scalar_mul(dg[:], identb[:], zi[:])
                for half in range(2):
                    tp = tpsum.tile([128, 4 * KM], f32)
                    for q in range(4):
                        tc_i = half * 4 + q
                        nc.tensor.matmul(
                            tp[:, q * KM : (q + 1) * KM],
                            et[:, tc_i * 128 : (tc_i + 1) * 128],
                            dg[:],
                            start=True,
                            stop=True,
                        )
                    dst = (
                        attnT[:]
                        .rearrange("p (tc gm) -> p tc gm", tc=NTC)[
                            :, half * 4 : (half + 1) * 4, g * KM : (g + 1) * KM
                        ]
                    )
                    nc.vector.tensor_copy(
                        dst, tp[:].rearrange("p (q m) -> p q m", q=4)
                    )

        # ---- phase 2: attn @ V  -> x^T ----
        attnT_v = attnT[:].rearrange(
            "p (tc g o lg) -> p tc o g lg", tc=NTC, g=NG, o=H
        )
        with ExitStack() as p2:
            xpsum = p2.enter_context(
                tc.tile_pool(name="xpsum", bufs=2, space="PSUM")
            )
            for h in range(H):
                xp = xpsum.tile([D, LCP], f32)
                for tc_i in range(NTC):
                    nc.tensor.matmul(
                        xp[:],
                        vsb[:, h * 512 + tc_i * D : h * 512 + (tc_i + 1) * D],
                        attnT_v[:, tc_i, h],
                        start=(tc_i == 0),
                        stop=(tc_i == NTC - 1),
                    )
                po = (h % 2) * D
                ko = (h // 2) * LCP
                nc.vector.tensor_copy(xT[po : po + D, ko : ko + LCP], xp[:])

            # ---- phase 3: proj -> out ----
            ppsum = p2.enter_context(
                tc.tile_pool(name="ppsum", bufs=2, space="PSUM")
            )
            ypool = p2.enter_context(tc.tile_pool(name="y", bufs=2))
            for lc in range(2):
                pp = ppsum.tile([128, 1024], f32)
                for hp in range(6):
                    lhs = xT[:, hp * LCP + lc * 128 : hp * LCP + (lc + 1) * 128]
                    nc.tensor.matmul(
                        pp[:, 0:512],
                        lhs,
                        pw[:, hp * DIM : hp * DIM + 512],
                        start=(hp == 0),
                        stop=(hp == 5),
                    )
                    nc.tensor.matmul(
                        pp[:, 512:768],
                        lhs,
                        pw[:, hp * DIM + 512 : hp * DIM + DIM],
                        start=(hp == 0),
                        stop=(hp == 5),
                    )
                yt = ypool.tile([128, DIM], f32)
                nc.vector.tensor_add(yt[:], pp[:, 0:DIM], pb[:])
                nc.scalar.dma_start(out_d[lc * 128 : (lc + 1) * 128, :], yt[:])

    nc.compile()
    return nc


def _host_prep(s0, s1, s2, v, fuse_w, fuse_b, proj_w, proj_b):
    """Build per-core input maps (bf16 packing; all transposes host-side)."""
    import ml_dtypes

    bf16 = ml_dtypes.bfloat16
    fuse_w = np.asarray(fuse_w, dtype=np.float32)
    fuse_b = np.asarray(fuse_b, dtype=np.float32)
    proj_w = np.asarray(proj_w, dtype=np.float32)
    proj_b = np.asarray(proj_b, dtype=np.float32)

    sb = [np.asarray(s, dtype=bf16) for s in (s0, s1, s2)]  # [B,12,L,T] bf16
    vb = np.asarray(v, dtype=bf16)  # [B,12,T,D]

    # block-diag conv weights: w_j[k=(lg,c), m=(o,lg)] = fuse_w[o, 12j+c]
    wts = np.zeros((3, KM, KM), dtype=bf16)
    for j in range(3):
        blk = fuse_w[:, 12 * j : 12 * (j + 1)].T.astype(bf16)  # [c, o]
        for lg in range(G):
            wts[j, lg * 12 : (lg + 1) * 12, lg::G] = blk
    b120 = np.repeat(fuse_b, G).astype(np.float32).reshape(KM, 1)  # p = o*G+lg
    identb = np.eye(KM, dtype=bf16)
    # pwp[p, hp*768+o] = proj_w[o, hp*128+p]
    pwp = np.ascontiguousarray(
        proj_w.T.reshape(6, 128, DIM).transpose(1, 0, 2).reshape(128, 6 * DIM)
    ).astype(bf16)
    pbb = np.broadcast_to(proj_b, (128, DIM)).copy()

    in_maps = []
    for k in range(NCORES):
        b = k // (NCORES // B)
        l0 = (k % (NCORES // B)) * LC
        # sc[g, j, lg*12+c, t] = s_j[b, c, l0+g*10+lg, t]  (l padded 256->260)
        sc = np.zeros((NG, 3, KM, T), dtype=bf16)
        core = np.stack([s[b, :, l0 : l0 + LC, :] for s in sb])  # [3,12,256,T]
        pad = np.zeros((3, 12, LCP - LC, T), dtype=bf16)
        corep = np.concatenate([core, pad], axis=2)  # [3,12,260,T]
        sc[:] = (
            corep.reshape(3, 12, NG, G, T)
            .transpose(2, 0, 3, 1, 4)
            .reshape(NG, 3, KM, T)
        )
        # vp[h, p, tc*64+d] = v[b, h, tc*128+p, d]
        vp = np.ascontiguousarray(
            vb[b].reshape(H, NTC, 128, D).transpose(0, 2, 1, 3).reshape(H, 128, NTC * D)
        )
        m = {
            "sc": sc,
            "vp": vp,
            "wts": wts,
            "b120": b120,
            "identb": identb,
            "pwp": pwp,
            "pbb": pbb,
        }
        in_maps.append(m)
    return in_maps


def _install_ntff_hook():
    """Provide antenv.axon_hooks (absent in this image) so trace=True works."""
    import os

    try:
        from antenv import axon_hooks  # noqa: F401

        return True
    except ImportError:
        pass
    try:
        import types
        import ctypes
        import contextlib
        import antenv

        so_path = "/opt/axon/libaxon_pjrt.so"
        if not os.path.exists(so_path):
            return False
        lib = ctypes.CDLL(so_path)
        if not hasattr(lib, "axon_start_nrt_profile"):
            return False
        lib.axon_start_nrt_profile.argtypes = [
            ctypes.POINTER(ctypes.c_int64),
            ctypes.c_size_t,
        ]
        lib.axon_start_nrt_profile.restype = ctypes.c_int64
        lib.axon_stop_nrt_profile.argtypes = [ctypes.c_char_p]
        lib.axon_stop_nrt_profile.restype = ctypes.c_int64

        @contextlib.contextmanager
        def _hook(output_dir, device_ids):
            import jax

            jax.devices()
            if device_ids:
                ids = (ctypes.c_int64 * len(device_ids))(*device_ids)
                rc = lib.axon_start_nrt_profile(ids, len(device_ids))
            else:
                rc = lib.axon_start_nrt_profile(None, 0)
            if rc != 0:
                raise RuntimeError(f"axon_start_nrt_profile rc={rc}")
            try:
                yield
            finally:
                n = lib.axon_stop_nrt_profile(str(output_dir).encode())
                print(f"ntff profile: {n} file(s) -> {output_dir}", file=sys.stderr)

        mod = types.ModuleType("antenv.axon_hooks")
        _h = {"hook": _hook}
        mod.set_axon_ntff_profile_hook = lambda h: _h.__setitem__("hook", h)
        mod.get_axon_ntff_profile_hook = lambda: _h["hook"]
        sys.modules["antenv.axon_hooks"] = mod
        antenv.axon_hooks = mod
        return True
    except Exception as e:  # degrade to untraced
        print("ntff hook install failed:", e, file=sys.stderr)
        return False


def kernel(s0, s1, s2, v, fuse_w, fuse_b, proj_w, proj_b, _trace=False):
    from concourse import bass_utils
    from concourse.bass_utils import run_bass_kernel_spmd

    if "nc" not in _CACHE:
        _CACHE["nc"] = _build_nc()
    nc = _CACHE["nc"]

    in_maps = _host_prep(s0, s1, s2, v, fuse_w, fuse_b, proj_w, proj_b)
    if _trace:
        _trace = _install_ntff_hook()
        bass_utils.upload_artifacts = lambda tmpdir: f"local:{tmpdir}"
    tmpdir = None
    if _trace:
        import tempfile

        tmpdir = tempfile.mkdtemp(prefix="bass_trace_")
        _CACHE["trace_dir"] = tmpdir
    try:
        res = run_bass_kernel_spmd(
            nc, in_maps, core_ids=list(range(NCORES)), trace=_trace, tmpdir=tmpdir
        )
    except Exception:
        if not _trace:
            raise
        import traceback

        traceback.print_exc()
        print("trace run failed; retrying untraced", file=sys.stderr)
        res = run_bass_kernel_spmd(nc, in_maps, core_ids=list(range(NCORES)))
    _CACHE["last_exec_time_ns"] = res.exec_time_ns
    _CACHE["last_results"] = res

    out = np.empty((B, L, DIM), dtype=np.float32)
    for k in range(NCORES):
        b = k // (NCORES // B)
        l0 = (k % (NCORES // B)) * LC
        out[b, l0 : l0 + LC, :] = res.results[k]["out"]
    return out


# revision 6
# speedup vs baseline: 1.8673x; 1.0425x over previous
"""Fused conv-attention kernel for Trainium2, sharded over 8 NeuronCores.

Reference computation (B=2, H=12, L=T=1024, D=64, FEA=3, DIM=768):
    scores = concat([s0,s1,s2], ch)            # [b, 36, l, t]
    fused  = einsum('bclt,oc->bolt', scores, fuse_w) + fuse_b
    attn   = softmax(fused, axis=-1)
    x      = einsum('bhlt,bhtd->bhld', attn, v)
    y      = merge_heads(x) @ proj_w.T + proj_b  # [b, l, 768]

Sharding: fully data-parallel over (b, l-block): core k handles b=k//4 and
l-rows [256*(k%4), 256*(k%4)+256).  Every op is local; no collectives.

All HBM traffic is bf16 (host casts/packs): scores 19.2MB/core with 2KB
descriptors so the DMA stripes across all 16 engines; v/proj weights flow on
the second HWDGE queue (scalar engine) in parallel with the score stream.

Per-core dataflow (G=10 l-rows per group, KM=120 partitions, LCP=260 padded):
  - conv as block-diag matmul: lhsT_j [120,120] holds fuse_w cols for score
    tensor j replicated block-diagonally over lg; j-outer loop -> 3 stationary
    loads per group, 6 matmuls of N=512 into PSUM [120,1024].
  - exp via ScalarE activation (bias=fuse_b, accum_out=row sums z); max
    subtraction skipped (|fused| <= ~5 so fp32 exp is safe).
  - softmax normalize FUSED into the PE transpose: stationary = et t-chunk,
    moving = diag(1/z) bf16 -> psum tp[t,120] holds normalized attn^T.
  - attnT sbuf layout [t-part, (tc, g, o, lg)] bf16: psum->sbuf copies are
    contiguous; phase-2 matmul rhs reads the strided (g,lg) view (N=260).
  - attn @ V: per (h, tc) matmul accumulating x^T [64, 260] over 8 t-chunks.
  - final proj: out[l,768] = sum_hp xT_hp^T @ pw_hp, bias added by DVE.
"""

import sys

import numpy as np

sys.path.insert(0, "/opt/trn_rl_repo")

B, H, L, T, D = 2, 12, 1024, 1024, 64
DIM = H * D  # 768
NCORES = 8
LC = L * B // NCORES  # 256 l-rows per core
G = 10  # l-rows per conv group
NG = 26  # groups per core (last is padded: 260 > 256)
LCP = NG * G  # 260
KM = 12 * G  # 120: conv matmul K and M
NTC = T // 128  # 8 t-chunks

_CACHE = {}


def _build_nc():
    import concourse.bacc as bacc
    import concourse.mybir as mybir
    import concourse.tile as tile
    from contextlib import ExitStack

    f32 = mybir.dt.float32
    bf16 = mybir.dt.bfloat16

    nc = bacc.Bacc(
        "TRN2", target_bir_lowering=False, debug=False, enable_asserts=False
    )

    sc_in = nc.dram_tensor("sc", [NG, KM, 3 * T], bf16, kind="ExternalInput").ap()
    v_in = nc.dram_tensor("vp", [H, 128, NTC * D], bf16, kind="ExternalInput").ap()
    w_in = nc.dram_tensor("wts", [3, KM, KM], bf16, kind="ExternalInput").ap()
    b_in = nc.dram_tensor("b120", [KM, 1], f32, kind="ExternalInput").ap()
    id_in = nc.dram_tensor("identb", [KM, KM], bf16, kind="ExternalInput").ap()
    pw_in = nc.dram_tensor("pwp", [128, 6 * DIM], bf16, kind="ExternalInput").ap()
    pb_in = nc.dram_tensor("pbb", [128, DIM], f32, kind="ExternalInput").ap()
    out_d = nc.dram_tensor("out", [LC, DIM], f32, kind="ExternalOutput").ap()

    with tile.TileContext(nc) as tc, ExitStack() as ctx:
        # ---- persistent SBUF (weights arrive on the scalar HWDGE queue) ----
        singles = ctx.enter_context(tc.tile_pool(name="singles", bufs=1))
        wt = [singles.tile([KM, KM], bf16, tag=f"wt{j}", name=f"wt{j}") for j in range(3)]
        for j in range(3):
            nc.scalar.dma_start(wt[j][:], w_in[j])
        b120 = singles.tile([KM, 1], f32)
        nc.scalar.dma_start(b120[:], b_in)
        identb = singles.tile([KM, KM], bf16)
        nc.scalar.dma_start(identb[:], id_in)
        vsb = singles.tile([128, H * NTC * D], bf16)  # [t%128, h*512 + tc*64 + d]
        for h in range(H):
            nc.scalar.dma_start(vsb[:, h * 512 : (h + 1) * 512], v_in[h])
        pw = singles.tile([128, 6 * DIM], bf16)  # [i%128, hp*768 + o]
        for q in range(3):
            nc.scalar.dma_start(
                pw[:, q * 1536 : (q + 1) * 1536], pw_in[:, q * 1536 : (q + 1) * 1536]
            )
        pb = singles.tile([128, DIM], f32)
        nc.scalar.dma_start(pb[:], pb_in)
        # attn^T accumulator: [t%128, tc*3120 + g*120 + (o*10+lg)]  bf16
        attnT = singles.tile([128, NTC * NG * KM], bf16)
        # x^T for proj: [i%128 part, (i//128)*260 + l]  bf16
        xT = singles.tile([128, 6 * LCP], bf16)

        # ---- phase 1: conv + softmax + normalized transpose, per group ----
        with ExitStack() as p1:
            spool = p1.enter_context(tc.tile_pool(name="scores", bufs=3))
            fpsum = p1.enter_context(
                tc.tile_pool(name="fpsum", bufs=2, space="PSUM")
            )
            epool = p1.enter_context(tc.tile_pool(name="exp", bufs=3))
            zpool = p1.enter_context(tc.tile_pool(name="z", bufs=4))
            dpool = p1.enter_context(tc.tile_pool(name="diag", bufs=2))
            tpsum = p1.enter_context(
                tc.tile_pool(name="tpsum", bufs=4, space="PSUM")
            )
            for g in range(NG):
                st = spool.tile([KM, 3 * T], bf16, tag="st", name=f"st_{g}")
                nc.sync.dma_start(st[:], sc_in[g])
                fp = fpsum.tile([KM, T], f32)
                for j in range(3):
                    for th in range(2):
                        nc.tensor.matmul(
                            fp[:, th * 512 : (th + 1) * 512],
                            wt[j][:],
                            st[:, j * T + th * 512 : j * T + (th + 1) * 512],
                            start=(j == 0),
                            stop=(j == 2),
                        )
                et = epool.tile([KM, T], bf16, tag="et")
                zt = zpool.tile([KM, 1], f32, tag="zt")
                nc.scalar.activation(
                    et[:],
                    fp[:],
                    mybir.ActivationFunctionType.Exp,
                    bias=b120[:],
                    accum_out=zt[:],
                )
                zi = zpool.tile([KM, 1], f32, tag="zi")
                nc.vector.reciprocal(zi[:], zt[:])
                dg = dpool.tile([KM, KM], bf16, tag="dg")
                nc.vector.tensor_scalar_mul(dg[:], identb[:], zi[:])
                for half in range(2):
                    tp = tpsum.tile([128, 4 * KM], f32)
                    for q in range(4):
                        tc_i = half * 4 + q
                        nc.tensor.matmul(
                            tp[:, q * KM : (q + 1) * KM],
                            et[:, tc_i * 128 : (tc_i + 1) * 128],
                            dg[:],
                            start=True,
                            stop=True,
                        )
                    dst = (
                        attnT[:]
                        .rearrange("p (tc gm) -> p tc gm", tc=NTC)[
                            :, half * 4 : (half + 1) * 4, g * KM : (g + 1) * KM
                        ]
                    )
                    nc.vector.tensor_copy(
                        dst, tp[:].rearrange("p (q m) -> p q m", q=4)
                    )

        # ---- phase 2: attn @ V  -> x^T ----
        attnT_v = attnT[:].rearrange(
            "p (tc g o lg) -> p tc o g lg", tc=NTC, g=NG, o=H
        )
        with ExitStack() as p2:
            xpsum = p2.enter_context(
                tc.tile_pool(name="xpsum", bufs=4, space="PSUM")
            )
            for h in range(H):
                xp = xpsum.tile([D, LCP], f32)
                for tc_i in range(NTC):
                    nc.tensor.matmul(
                        xp[:],
                        vsb[:, h * 512 + tc_i * D : h * 512 + (tc_i + 1) * D],
                        attnT_v[:, tc_i, h],
                        start=(tc_i == 0),
                        stop=(tc_i == NTC - 1),
                    )
                po = (h % 2) * D
                ko = (h // 2) * LCP
                nc.vector.tensor_copy(xT[po : po + D, ko : ko + LCP], xp[:])

            # ---- phase 3: proj -> out ----
            ppsum = p2.enter_context(
                tc.tile_pool(name="ppsum", bufs=2, space="PSUM")
            )
            ypool = p2.enter_context(tc.tile_pool(name="y", bufs=2))
            for lc in range(2):
                pp = ppsum.tile([128, 1024], f32)
                for hp in range(6):
                    lhs = xT[:, hp * LCP + lc * 128 : hp * LCP + (lc + 1) * 128]
                    nc.tensor.matmul(
                        pp[:, 0:512],
                        lhs,
                        pw[:, hp * DIM : hp * DIM + 512],
                        start=(hp == 0),
                        stop=(hp == 5),
                    )
                    nc.tensor.matmul(
                        pp[:, 512:768],
                        lhs,
                        pw[:, hp * DIM + 512 : hp * DIM + DIM],
                        start=(hp == 0),
                        stop=(hp == 5),
                    )
                yt = ypool.tile([128, DIM], f32)
                nc.vector.tensor_add(yt[:], pp[:, 0:DIM], pb[:])
                nc.scalar.dma_start(out_d[lc * 128 : (lc + 1) * 128, :], yt[:])

    nc.compile()
    return nc


def _host_prep(s0, s1, s2, v, fuse_w, fuse_b, proj_w, proj_b):
    """Build per-core input maps (bf16 packing; all transposes host-side)."""
    import ml_dtypes

    bf16 = ml_dtypes.bfloat16
    fuse_w = np.asarray(fuse_w, dtype=np.float32)
    fuse_b = np.asarray(fuse_b, dtype=np.float32)
    proj_w = np.asarray(proj_w, dtype=np.float32)
    proj_b = np.asarray(proj_b, dtype=np.float32)

    sb = [np.asarray(s, dtype=bf16) for s in (s0, s1, s2)]  # [B,12,L,T] bf16
    vb = np.asarray(v, dtype=bf16)  # [B,12,T,D]

    # block-diag conv weights: w_j[k=(lg,c), m=(o,lg)] = fuse_w[o, 12j+c]
    wts = np.zeros((3, KM, KM), dtype=bf16)
    for j in range(3):
        blk = fuse_w[:, 12 * j : 12 * (j + 1)].T.astype(bf16)  # [c, o]
        for lg in range(G):
            wts[j, lg * 12 : (lg + 1) * 12, lg::G] = blk
    b120 = np.repeat(fuse_b, G).astype(np.float32).reshape(KM, 1)  # p = o*G+lg
    identb = np.eye(KM, dtype=bf16)
    # pwp[p, hp*768+o] = proj_w[o, hp*128+p]
    pwp = np.ascontiguousarray(
        proj_w.T.reshape(6, 128, DIM).transpose(1, 0, 2).reshape(128, 6 * DIM)
    ).astype(bf16)
    pbb = np.broadcast_to(proj_b, (128, DIM)).copy()

    in_maps = []
    for k in range(NCORES):
        b = k // (NCORES // B)
        l0 = (k % (NCORES // B)) * LC
        # sc[g, lg*12+c, j*T+t] = s_j[b, c, l0+g*10+lg, t]  (l padded 256->260)
        core = np.stack([s[b, :, l0 : l0 + LC, :] for s in sb])  # [3,12,256,T]
        pad = np.zeros((3, 12, LCP - LC, T), dtype=bf16)
        corep = np.concatenate([core, pad], axis=2)  # [3,12,260,T]
        sc = np.ascontiguousarray(
            corep.reshape(3, 12, NG, G, T)
            .transpose(2, 3, 1, 0, 4)
            .reshape(NG, KM, 3 * T)
        )
        # vp[h, p, tc*64+d] = v[b, h, tc*128+p, d]
        vp = np.ascontiguousarray(
            vb[b].reshape(H, NTC, 128, D).transpose(0, 2, 1, 3).reshape(H, 128, NTC * D)
        )
        m = {
            "sc": sc,
            "vp": vp,
            "wts": wts,
            "b120": b120,
            "identb": identb,
            "pwp": pwp,
            "pbb": pbb,
        }
        in_maps.append(m)
    return in_maps


def _install_ntff_hook():
    """Provide antenv.axon_hooks (absent in this image) so trace=True works."""
    import os

    try:
        from antenv import axon_hooks  # noqa: F401

        return True
    except ImportError:
        pass
    try:
        import types
        import ctypes
        import contextlib
        import antenv

        so_path = "/opt/axon/libaxon_pjrt.so"
        if not os.path.exists(so_path):
            return False
        lib = ctypes.CDLL(so_path)
        if not hasattr(lib, "axon_start_nrt_profile"):
            return False
        lib.axon_start_nrt_profile.argtypes = [
            ctypes.POINTER(ctypes.c_int64),
            ctypes.c_size_t,
        ]
        lib.axon_start_nrt_profile.restype = ctypes.c_int64
        lib.axon_stop_nrt_profile.argtypes = [ctypes.c_char_p]
        lib.axon_stop_nrt_profile.restype = ctypes.c_int64

        @contextlib.contextmanager
        def _hook(output_dir, device_ids):
            import jax

            jax.devices()
            if device_ids:
                ids = (ctypes.c_int64 * len(device_ids))(*device_ids)
                rc = lib.axon_start_nrt_profile(ids, len(device_ids))
            else:
                rc = lib.axon_start_nrt_profile(None, 0)
            if rc != 0:
                raise RuntimeError(f"axon_start_nrt_profile rc={rc}")
            try:
                yield
            finally:
                n = lib.axon_stop_nrt_profile(str(output_dir).encode())
                print(f"ntff profile: {n} file(s) -> {output_dir}", file=sys.stderr)

        mod = types.ModuleType("antenv.axon_hooks")
        _h = {"hook": _hook}
        mod.set_axon_ntff_profile_hook = lambda h: _h.__setitem__("hook", h)
        mod.get_axon_ntff_profile_hook = lambda: _h["hook"]
        sys.modules["antenv.axon_hooks"] = mod
        antenv.axon_hooks = mod
        return True
    except Exception as e:  # degrade to untraced
        print("ntff hook install failed:", e, file=sys.stderr)
        return False


def kernel(s0, s1, s2, v, fuse_w, fuse_b, proj_w, proj_b, _trace=False):
    from concourse import bass_utils
    from concourse.bass_utils import run_bass_kernel_spmd

    if "nc" not in _CACHE:
        _CACHE["nc"] = _build_nc()
    nc = _CACHE["nc"]

    in_maps = _host_prep(s0, s1, s2, v, fuse_w, fuse_b, proj_w, proj_b)
    if _trace:
        _trace = _install_ntff_hook()
        bass_utils.upload_artifacts = lambda tmpdir: f"local:{tmpdir}"
    tmpdir = None
    if _trace:
        import tempfile

        tmpdir = tempfile.mkdtemp(prefix="bass_trace_")
        _CACHE["trace_dir"] = tmpdir
    try:
        res = run_bass_kernel_spmd(
            nc, in_maps, core_ids=list(range(NCORES)), trace=_trace, tmpdir=tmpdir
        )
    except Exception:
        if not _trace:
            raise
        import traceback

        traceback.print_exc()
        print("trace run failed; retrying untraced", file=sys.stderr)
        res = run_bass_kernel_spmd(nc, in_maps, core_ids=list(range(NCORES)))
    _CACHE["last_exec_time_ns"] = res.exec_time_ns
    _CACHE["last_results"] = res

    out = np.empty((B, L, DIM), dtype=np.float32)
    for k in range(NCORES):
        b = k // (NCORES // B)
        l0 = (k % (NCORES // B)) * LC
        out[b, l0 : l0 + LC, :] = res.results[k]["out"]
    return out


# revision 10
# speedup vs baseline: 1.8863x; 1.0102x over previous
"""Fused conv-attention kernel for Trainium2, sharded over 8 NeuronCores.

Reference computation (B=2, H=12, L=T=1024, D=64, FEA=3, DIM=768):
    scores = concat([s0,s1,s2], ch)            # [b, 36, l, t]
    fused  = einsum('bclt,oc->bolt', scores, fuse_w) + fuse_b
    attn   = softmax(fused, axis=-1)
    x      = einsum('bhlt,bhtd->bhld', attn, v)
    y      = merge_heads(x) @ proj_w.T + proj_b  # [b, l, 768]

Sharding: fully data-parallel over (b, l-block): core k handles b=k//4 and
l-rows [256*(k%4), 256*(k%4)+256).  Every op is local; no collectives.

All HBM traffic is bf16 (host casts/packs): scores 19.2MB/core with 2KB
descriptors so the DMA stripes across all 16 engines; v/proj weights flow on
the second HWDGE queue (scalar engine) in parallel with the score stream.

Per-core dataflow (G=10 l-rows per group, KM=120 partitions, LCP=260 padded):
  - conv as block-diag matmul: lhsT_j [120,120] holds fuse_w cols for score
    tensor j replicated block-diagonally over lg; j-outer loop -> 3 stationary
    loads per group, 6 matmuls of N=512 into PSUM [120,1024].
  - exp via ScalarE activation (bias=fuse_b, accum_out=row sums z); max
    subtraction skipped (|fused| <= ~5 so fp32 exp is safe).
  - softmax normalize FUSED into the PE transpose: stationary = et t-chunk,
    moving = diag(1/z) bf16 -> psum tp[t,120] holds normalized attn^T.
  - attnT sbuf layout [t-part, (tc, g, o, lg)] bf16: psum->sbuf copies are
    contiguous; phase-2 matmul rhs reads the strided (g,lg) view (N=260).
  - attn @ V: per (h, tc) matmul accumulating x^T [64, 260] over 8 t-chunks.
  - final proj: out[l,768] = sum_hp xT_hp^T @ pw_hp, bias added by DVE.
"""

import sys

import numpy as np

sys.path.insert(0, "/opt/trn_rl_repo")

B, H, L, T, D = 2, 12, 1024, 1024, 64
DIM = H * D  # 768
NCORES = 8
LC = L * B // NCORES  # 256 l-rows per core
G = 10  # l-rows per conv group
NG = 26  # groups per core (last is padded: 260 > 256)
LCP = NG * G  # 260
KM = 12 * G  # 120: conv matmul K and M
NTC = T // 128  # 8 t-chunks

_CACHE = {}


def _build_nc():
    import concourse.bacc as bacc
    import concourse.mybir as mybir
    import concourse.tile as tile
    from contextlib import ExitStack

    f32 = mybir.dt.float32
    bf16 = mybir.dt.bfloat16

    nc = bacc.Bacc(
        "TRN2", target_bir_lowering=False, debug=False, enable_asserts=False
    )

    # big streams are DMA'd with f32-typed descriptors (the DMA engines move
    # 4-byte elements faster than 2-byte ones); compute reads bf16 bitcasts
    sc_in = nc.dram_tensor("sc", [NG, KM, 3 * T // 2], f32, kind="ExternalInput").ap()
    v_in = nc.dram_tensor("vp", [H, 128, NTC * D // 2], f32, kind="ExternalInput").ap()
    w_in = nc.dram_tensor("wts", [3, KM, KM], bf16, kind="ExternalInput").ap()
    b_in = nc.dram_tensor("b120", [KM, 1], f32, kind="ExternalInput").ap()
    id_in = nc.dram_tensor("identb", [KM, KM], bf16, kind="ExternalInput").ap()
    pw_in = nc.dram_tensor("pwp", [128, 3 * DIM], f32, kind="ExternalInput").ap()
    pb_in = nc.dram_tensor("pbb", [128, DIM], f32, kind="ExternalInput").ap()
    out_d = nc.dram_tensor("out", [LC, DIM], f32, kind="ExternalOutput").ap()

    with tile.TileContext(nc) as tc, ExitStack() as ctx:
        # ---- persistent SBUF (weights arrive on the scalar HWDGE queue) ----
        singles = ctx.enter_context(tc.tile_pool(name="singles", bufs=1))
        wt = [singles.tile([KM, KM], bf16, tag=f"wt{j}", name=f"wt{j}") for j in range(3)]
        for j in range(3):
            nc.scalar.dma_start(wt[j][:], w_in[j])
        b120 = singles.tile([KM, 1], f32)
        nc.scalar.dma_start(b120[:], b_in)
        identb = singles.tile([KM, KM], bf16)
        nc.scalar.dma_start(identb[:], id_in)
        vsb_f = singles.tile([128, H * NTC * D // 2], f32)
        for h in range(H):
            nc.scalar.dma_start(vsb_f[:, h * 256 : (h + 1) * 256], v_in[h])
        vsb = vsb_f[:].bitcast(bf16)  # [t%128, h*512 + tc*64 + d]
        pw_f = singles.tile([128, 3 * DIM], f32)
        for q in range(3):
            nc.scalar.dma_start(
                pw_f[:, q * 768 : (q + 1) * 768], pw_in[:, q * 768 : (q + 1) * 768]
            )
        pw = pw_f[:].bitcast(bf16)  # [i%128, hp*768 + o]
        pb = singles.tile([128, DIM], f32)
        nc.scalar.dma_start(pb[:], pb_in)
        # attn^T accumulator: [t%128, tc*3120 + g*120 + (o*10+lg)]  bf16
        attnT = singles.tile([128, NTC * NG * KM], bf16)
        # x^T for proj: [i%128 part, (i//128)*260 + l]  bf16
        xT = singles.tile([128, 6 * LCP], bf16)

        # ---- phase 1: conv + softmax + normalized transpose, per group ----
        with ExitStack() as p1:
            spool = p1.enter_context(tc.tile_pool(name="scores", bufs=3))
            fpsum = p1.enter_context(
                tc.tile_pool(name="fpsum", bufs=2, space="PSUM")
            )
            epool = p1.enter_context(tc.tile_pool(name="exp", bufs=3))
            zpool = p1.enter_context(tc.tile_pool(name="z", bufs=4))
            dpool = p1.enter_context(tc.tile_pool(name="diag", bufs=2))
            tpsum = p1.enter_context(
                tc.tile_pool(name="tpsum", bufs=4, space="PSUM")
            )
            for g in range(NG):
                st_f = spool.tile([KM, 3 * T // 2], f32, tag="st", name=f"st_{g}")
                nc.sync.dma_start(st_f[:], sc_in[g])
                st = st_f[:].bitcast(bf16)
                fp = fpsum.tile([KM, T], f32)
                for j in range(3):
                    for th in range(2):
                        nc.tensor.matmul(
                            fp[:, th * 512 : (th + 1) * 512],
                            wt[j][:],
                            st[:, j * T + th * 512 : j * T + (th + 1) * 512],
                            start=(j == 0),
                            stop=(j == 2),
                        )
                et = epool.tile([KM, T], bf16, tag="et")
                zt = zpool.tile([KM, 1], f32, tag="zt")
                nc.scalar.activation(
                    et[:],
                    fp[:],
                    mybir.ActivationFunctionType.Exp,
                    bias=b120[:],
                    accum_out=zt[:],
                )
                zi = zpool.tile([KM, 1], f32, tag="zi")
                nc.vector.reciprocal(zi[:], zt[:])
                dg = dpool.tile([KM, KM], bf16, tag="dg")
                nc.vector.tensor_scalar_mul(dg[:], identb[:], zi[:])
                for half in range(2):
                    tp = tpsum.tile([128, 4 * KM], f32)
                    for q in range(4):
                        tc_i = half * 4 + q
                        nc.tensor.matmul(
                            tp[:, q * KM : (q + 1) * KM],
                            et[:, tc_i * 128 : (tc_i + 1) * 128],
                            dg[:],
                            start=True,
                            stop=True,
                        )
                    dst = (
                        attnT[:]
                        .rearrange("p (tc gm) -> p tc gm", tc=NTC)[
                            :, half * 4 : (half + 1) * 4, g * KM : (g + 1) * KM
                        ]
                    )
                    nc.vector.tensor_copy(
                        dst, tp[:].rearrange("p (q m) -> p q m", q=4)
                    )

        # ---- phase 2: attn @ V  -> x^T ----
        attnT_v = attnT[:].rearrange(
            "p (tc g o lg) -> p tc o g lg", tc=NTC, g=NG, o=H
        )
        with ExitStack() as p2:
            xpsum = p2.enter_context(
                tc.tile_pool(name="xpsum", bufs=4, space="PSUM")
            )
            for h in range(H):
                xp = xpsum.tile([D, LCP], f32)
                for tc_i in range(NTC):
                    nc.tensor.matmul(
                        xp[:],
                        vsb[:, h * 512 + tc_i * D : h * 512 + (tc_i + 1) * D],
                        attnT_v[:, tc_i, h],
                        start=(tc_i == 0),
                        stop=(tc_i == NTC - 1),
                    )
                po = (h % 2) * D
                ko = (h // 2) * LCP
                nc.vector.tensor_copy(xT[po : po + D, ko : ko + LCP], xp[:])

            # ---- phase 3: proj -> out ----
            ppsum = p2.enter_context(
                tc.tile_pool(name="ppsum", bufs=2, space="PSUM")
            )
            ypool = p2.enter_context(tc.tile_pool(name="y", bufs=2))
            for lc in range(2):
                pp = ppsum.tile([128, 1024], f32)
                for hp in range(6):
                    lhs = xT[:, hp * LCP + lc * 128 : hp * LCP + (lc + 1) * 128]
                    nc.tensor.matmul(
                        pp[:, 0:512],
                        lhs,
                        pw[:, hp * DIM : hp * DIM + 512],
                        start=(hp == 0),
                        stop=(hp == 5),
                    )
                    nc.tensor.matmul(
                        pp[:, 512:768],
                        lhs,
                        pw[:, hp * DIM + 512 : hp * DIM + DIM],
                        start=(hp == 0),
                        stop=(hp == 5),
                    )
                yt = ypool.tile([128, DIM], f32)
                nc.vector.tensor_add(yt[:], pp[:, 0:DIM], pb[:])
                nc.scalar.dma_start(out_d[lc * 128 : (lc + 1) * 128, :], yt[:])

    nc.compile()
    return nc


def _host_prep(s0, s1, s2, v, fuse_w, fuse_b, proj_w, proj_b):
    """Build per-core input maps (bf16 packing; all transposes host-side)."""
    import ml_dtypes

    bf16 = ml_dtypes.bfloat16
    fuse_w = np.asarray(fuse_w, dtype=np.float32)
    fuse_b = np.asarray(fuse_b, dtype=np.float32)
    proj_w = np.asarray(proj_w, dtype=np.float32)
    proj_b = np.asarray(proj_b, dtype=np.float32)

    sb = [np.asarray(s, dtype=bf16) for s in (s0, s1, s2)]  # [B,12,L,T] bf16
    vb = np.asarray(v, dtype=bf16)  # [B,12,T,D]

    # block-diag conv weights: w_j[k=(lg,c), m=(o,lg)] = fuse_w[o, 12j+c]
    wts = np.zeros((3, KM, KM), dtype=bf16)
    for j in range(3):
        blk = fuse_w[:, 12 * j : 12 * (j + 1)].T.astype(bf16)  # [c, o]
        for lg in range(G):
            wts[j, lg * 12 : (lg + 1) * 12, lg::G] = blk
    b120 = np.repeat(fuse_b, G).astype(np.float32).reshape(KM, 1)  # p = o*G+lg
    identb = np.eye(KM, dtype=bf16)
    # pwp[p, hp*768+o] = proj_w[o, hp*128+p]
    pwp = np.ascontiguousarray(
        proj_w.T.reshape(6, 128, DIM).transpose(1, 0, 2).reshape(128, 6 * DIM)
    ).astype(bf16)
    pbb = np.broadcast_to(proj_b, (128, DIM)).copy()

    in_maps = []
    for k in range(NCORES):
        b = k // (NCORES // B)
        l0 = (k % (NCORES // B)) * LC
        # sc[g, lg*12+c, j*T+t] = s_j[b, c, l0+g*10+lg, t]  (l padded 256->260)
        core = np.stack([s[b, :, l0 : l0 + LC, :] for s in sb])  # [3,12,256,T]
        pad = np.zeros((3, 12, LCP - LC, T), dtype=bf16)
        corep = np.concatenate([core, pad], axis=2)  # [3,12,260,T]
        sc = np.ascontiguousarray(
            corep.reshape(3, 12, NG, G, T)
            .transpose(2, 3, 1, 0, 4)
            .reshape(NG, KM, 3 * T)
        )
        # vp[h, p, tc*64+d] = v[b, h, tc*128+p, d]
        vp = np.ascontiguousarray(
            vb[b].reshape(H, NTC, 128, D).transpose(0, 2, 1, 3).reshape(H, 128, NTC * D)
        )
        m = {
            "sc": sc.view(np.float32),
            "vp": vp.view(np.float32),
            "wts": wts,
            "b120": b120,
            "identb": identb,
            "pwp": pwp.view(np.float32),
            "pbb": pbb,
        }
        in_maps.append(m)
    return in_maps


def _install_ntff_hook():
    """Provide antenv.axon_hooks (absent in this image) so trace=True works."""
    import os

    try:
        from antenv import axon_hooks  # noqa: F401

        return True
    except ImportError:
        pass
    try:
        import types
        import ctypes
        import contextlib
        import antenv

        so_path = "/opt/axon/libaxon_pjrt.so"
        if not os.path.exists(so_path):
            return False
        lib = ctypes.CDLL(so_path)
        if not hasattr(lib, "axon_start_nrt_profile"):
            return False
        lib.axon_start_nrt_profile.argtypes = [
            ctypes.POINTER(ctypes.c_int64),
            ctypes.c_size_t,
        ]
        lib.axon_start_nrt_profile.restype = ctypes.c_int64
        lib.axon_stop_nrt_profile.argtypes = [ctypes.c_char_p]
        lib.axon_stop_nrt_profile.restype = ctypes.c_int64

        @contextlib.contextmanager
        def _hook(output_dir, device_ids):
            import jax

            jax.devices()
            if device_ids:
                ids = (ctypes.c_int64 * len(device_ids))(*device_ids)
                rc = lib.axon_start_nrt_profile(ids, len(device_ids))
            else:
                rc = lib.axon_start_nrt_profile(None, 0)
            if rc != 0:
                raise RuntimeError(f"axon_start_nrt_profile rc={rc}")
            try:
                yield
            finally:
                n = lib.axon_stop_nrt_profile(str(output_dir).encode())
                print(f"ntff profile: {n} file(s) -> {output_dir}", file=sys.stderr)

        mod = types.ModuleType("antenv.axon_hooks")
        _h = {"hook": _hook}
        mod.set_axon_ntff_profile_hook = lambda h: _h.__setitem__("hook", h)
        mod.get_axon_ntff_profile_hook = lambda: _h["hook"]
        sys.modules["antenv.axon_hooks"] = mod
        antenv.axon_hooks = mod
        return True
    except Exception as e:  # degrade to untraced
        print("ntff hook install failed:", e, file=sys.stderr)
        return False


def kernel(s0, s1, s2, v, fuse_w, fuse_b, proj_w, proj_b, _trace=False):
    from concourse import bass_utils
    from concourse.bass_utils import run_bass_kernel_spmd

    if "nc" not in _CACHE:
        _CACHE["nc"] = _build_nc()
    nc = _CACHE["nc"]

    in_maps = _host_prep(s0, s1, s2, v, fuse_w, fuse_b, proj_w, proj_b)
    if _trace:
        _trace = _install_ntff_hook()
        bass_utils.upload_artifacts = lambda tmpdir: f"local:{tmpdir}"
    tmpdir = None
    if _trace:
        import tempfile

        tmpdir = tempfile.mkdtemp(prefix="bass_trace_")
        _CACHE["trace_dir"] = tmpdir
    try:
        res = run_bass_kernel_spmd(
            nc, in_maps, core_ids=list(range(NCORES)), trace=_trace, tmpdir=tmpdir
        )
    except Exception:
        if not _trace:
            raise
        import traceback

        traceback.print_exc()
        print("trace run failed; retrying untraced", file=sys.stderr)
        res = run_bass_kernel_spmd(nc, in_maps, core_ids=list(range(NCORES)))
    _CACHE["last_exec_time_ns"] = res.exec_time_ns
    _CACHE["last_results"] = res

    out = np.empty((B, L, DIM), dtype=np.float32)
    for k in range(NCORES):
        b = k // (NCORES // B)
        l0 = (k % (NCORES // B)) * LC
        out[b, l0 : l0 + LC, :] = res.results[k]["out"]
    return out
